# revision 13
# baseline (speedup 1.0000x reference)
"""Fused ArcFace + batch-hard-triplet loss on 8 TRN2 NeuronCores — v10.

Per core (class-shard 6656-padded, batch sorted by label on host):
  - exp drain: ScalarE native exp (accum_out) for NA=9 tiles/bt; DVE
    fp16-Schraudolph for 4 tiles/bt (TS add+max -> int16, then STT
    min/min accum over the fp16-bitcast view at 2x).
  - W normalized on device (52 STT norms + Newton rsqrt + batched TT
    scale) then ONE batched SBUF->SBUF DMA block-transpose.
  - triplet: labels sorted + per-core column permutation (own-neighborhood
    first, wrapped) makes same-label bands static windows ->
    TENSOR_MASK_REDUCE windows for hardest-pos, full-width inverted for
    hardest-neg.
"""
import math
import os
import sys
from contextlib import ExitStack

import numpy as np

for _p in ("/opt/trn_rl_repo", os.path.expanduser("~/.axon_site/_ro/trn_rl_repo")):
    if _p not in sys.path and os.path.isdir(_p):
        sys.path.insert(0, _p)

import ml_dtypes

B, D, C = 2048, 128, 50000
NCORES = 8
CSH = C // NCORES            # 6250
CPAD = 6656                  # 52 tiles of 128 = 13 x 512
NWT = CPAD // 128            # 52
NBT = B // 128               # 16
RB = B // NCORES             # 256
ARC_SCALE = 64.0
ARC_MARGIN = 0.5
COS_M, SIN_M = math.cos(ARC_MARGIN), math.sin(ARC_MARGIN)
TH = math.cos(math.pi - ARC_MARGIN)
MMc = math.sin(math.pi - ARC_MARGIN) * ARC_MARGIN
LABEL_SMOOTH = 0.1
TRIPLET_MARGIN = 0.3
W_ARC, W_TRI = 1.0, 0.5

A_H = 2.0 ** 10 / math.log(2.0)
DELTA = 18.0
C_H = 60.0
S1 = 15.0 * 1024 - C_H - A_H * DELTA
EDELTA = math.exp(DELTA)

NA = int(os.environ.get("KERNEL_NA", "10"))          # ACT tiles per bt (of 13)
ACT_GROUPS = {7: [3, 2, 2], 8: [3, 3, 2], 9: [3, 3, 3], 10: [3, 3, 2, 2]}[NA]
DVE_GROUPS = {3: [2, 1], 4: [2, 2], 5: [2, 2, 1], 6: [2, 2, 2]}[13 - NA]

_CACHE = {}


def _build_nc():
    import concourse.bass as bass
    from concourse import bacc, mybir, tile
    from concourse.dve_ops import TENSOR_MASK_REDUCE

    f32 = mybir.dt.float32
    bf16 = mybir.dt.bfloat16
    fp16 = mybir.dt.float16
    i16 = mybir.dt.int16
    A = mybir.AluOpType
    AF = mybir.ActivationFunctionType
    X = mybir.AxisListType.X

    nc = bacc.Bacc("TRN2", target_bir_lowering=False, debug=False,
                   num_devices=NCORES)

    wsh = nc.dram_tensor("wsh", [CPAD, D], bf16, kind="ExternalInput").ap()
    embTs_d = nc.dram_tensor("embTs", [128, B], bf16, kind="ExternalInput").ap()
    embT_d = nc.dram_tensor("embT", [128, B], bf16, kind="ExternalInput").ap()
    embTt_d = nc.dram_tensor("embTt", [128, B], bf16, kind="ExternalInput").ap()
    embBT2_d = nc.dram_tensor("embBT2", [128, RB], bf16, kind="ExternalInput").ap()
    ssbt_d = nc.dram_tensor("ssbt", [B], bf16, kind="ExternalInput").ap()
    ssB_d = nc.dram_tensor("ssB", [128, 2], f32, kind="ExternalInput").ap()
    stend_d = nc.dram_tensor("stend", [128, 8], f32, kind="ExternalInput").ap()
    rinv_d = nc.dram_tensor("rinv16", [128, NBT], f32, kind="ExternalInput").ap()

    o_se = nc.dram_tensor("sumexp", [128, NBT], f32, kind="ExternalOutput").ap()
    o_sc = nc.dram_tensor("sumcos", [128, NBT], f32, kind="ExternalOutput").ap()
    o_tri = nc.dram_tensor("tri", [128, 4], f32, kind="ExternalOutput").ap()

    with tile.TileContext(nc) as tc, ExitStack() as ctx:
        sing = ctx.enter_context(tc.tile_pool(name="sing", bufs=1))
        tmp = ctx.enter_context(tc.tile_pool(name="tmp", bufs=2))
        n16p = ctx.enter_context(tc.tile_pool(name="n16p", bufs=2))
        psA = ctx.enter_context(tc.tile_pool(name="psA", bufs=2, space="PSUM"))
        psD = ctx.enter_context(tc.tile_pool(name="psD", bufs=1, space="PSUM"))

        # ---------------- input DMAs
        embTs = sing.tile([128, B], bf16)
        nc.sync.dma_start(out=embTs, in_=embTs_d)
        embT = sing.tile([128, B], bf16)
        nc.sync.dma_start(out=embT, in_=embT_d)
        embTt = sing.tile([128, B], bf16)
        nc.sync.dma_start(out=embTt, in_=embTt_d)
        embBT2 = sing.tile([128, RB], bf16)
        nc.sync.dma_start(out=embBT2, in_=embBT2_d)
        SQB = sing.tile([128, B], bf16)
        nc.sync.dma_start(out=SQB, in_=ssbt_d.partition_broadcast(128))
        ssB = sing.tile([128, 2], f32)
        nc.sync.dma_start(out=ssB, in_=ssB_d)
        stend = sing.tile([128, 8], f32)
        nc.sync.dma_start(out=stend, in_=stend_d)
        rinv16 = sing.tile([128, NBT], f32)
        nc.sync.dma_start(out=rinv16, in_=rinv_d)

        # ---------------- W prep, pipelined in 4 tile-groups
        WG = [(0, 18), (18, 18), (36, 8), (44, 8)]
        wnat = sing.tile([128, NWT, 128], bf16)
        wsrc = wsh.rearrange("(t p) d -> p t d", p=128)
        wn = sing.tile([128, NWT, 128], bf16)
        wTn = sing.tile([128, CPAD], bf16)
        ssw = sing.tile([128, NWT], f32)
        rw = sing.tile([128, NWT], f32)

        def wprep_group(t0, gn):
            sl = slice(t0, t0 + gn)
            nc.sync.dma_start(out=wnat[:, sl, :], in_=wsrc[:, sl, :])
            sq = tmp.tile([128, 18, 128], bf16, tag="sq")
            nc.vector.tensor_tensor(out=sq[:, :gn, :], in0=wnat[:, sl, :],
                                    in1=wnat[:, sl, :], op=A.mult)
            nc.vector.tensor_reduce(out=ssw[:, sl], in_=sq[:, :gn, :], axis=X,
                                    op=A.add)
            nc.vector.memset(rw[:, sl], 14.0)
            for it in range(3):
                t1 = tmp.tile([128, 18], f32, tag="nt1")
                nc.vector.tensor_tensor(out=t1[:, :gn], in0=rw[:, sl],
                                        in1=rw[:, sl], op=A.mult)
                nc.vector.tensor_tensor(out=t1[:, :gn], in0=t1[:, :gn],
                                        in1=ssw[:, sl], op=A.mult)
                nc.vector.tensor_scalar(out=t1[:, :gn], in0=t1[:, :gn],
                                        scalar1=-0.5, scalar2=1.5,
                                        op0=A.mult, op1=A.add)
                nc.vector.tensor_tensor(out=rw[:, sl], in0=rw[:, sl],
                                        in1=t1[:, :gn], op=A.mult)
            rwb = rw[:, sl].to_broadcast((128, gn, 128))
            nc.vector.tensor_tensor(out=wn[:, sl, :], in0=wnat[:, sl, :],
                                    in1=rwb, op=A.mult)
            nc.sync.dma_start_transpose(
                out=wTn[:, 128 * t0:128 * (t0 + gn)].rearrange(
                    "a (t p) -> a t p", p=128),
                in_=wn[:, sl, :].rearrange("a t d -> a (t d)"))

        wprep_group(*WG[0])
        wprep_group(*WG[1])

        # ---------------- triplet (own-neighborhood-permuted columns)
        d2p = [sing.tile([128, B], bf16, name=f"d2p{k}") for k in range(2)]
        d2n = [sing.tile([128, B], bf16, name=f"d2n{k}") for k in range(2)]
        mrj = sing.tile([128, B], bf16)
        mrw = sing.tile([128, 256], bf16)
        tri_acc = sing.tile([128, 4], f32)

        def tri_mms(k):
            pa = psA.tile([128, 1536], f32, tag="pa")
            for j in range(3):
                nc.tensor.matmul(pa[:, 512 * j:512 * j + 512],
                                 embBT2[:, 128 * k:128 * k + 128],
                                 embTt[:, 512 * j:512 * j + 512],
                                 start=True, stop=True)
            pd = psD.tile([128, 1024], f32, tag="pd")
            nc.tensor.matmul(pd[:, 0:512], embBT2[:, 128 * k:128 * k + 128],
                             embTt[:, 1536:2048], start=True, stop=True)
            nc.vector.scalar_tensor_tensor(out=d2p[k][:, :1536], in0=pa,
                                           scalar=1.0, in1=SQB[:, :1536],
                                           op0=A.mult, op1=A.add)
            nc.vector.scalar_tensor_tensor(out=d2p[k][:, 1536:], in0=pd[:, 0:512],
                                           scalar=1.0, in1=SQB[:, 1536:],
                                           op0=A.mult, op1=A.add)
            nc.gpsimd.tensor_scalar(out=d2n[k], in0=d2p[k], scalar1=-1.0,
                                    scalar2=None, op0=A.mult)

        def tri_hp(k):
            # hp^2 - ss_k = max over window [st,en) of d2pos; window = [128k, 128k+256)
            nc.vector._custom_dve(TENSOR_MASK_REDUCE, out=mrw,
                                  in0=d2p[k][:, 128 * k:128 * k + 256],
                                  in1=stend[:, 2 * k + 1:2 * k + 2],
                                  s0=stend[:, 2 * k:2 * k + 1], s1=-3.0e38,
                                  imm2=1.0, accum_out=tri_acc[:, 2 * k:2 * k + 1])

        def tri_hn(k):
            # ss_k - hn^2 = max over NOT [st,en) of -d2pos (global perm coords)
            nc.vector._custom_dve(TENSOR_MASK_REDUCE, out=mrj, in0=d2n[k],
                                  in1=stend[:, 4 + 2 * k:5 + 2 * k],
                                  s0=stend[:, 5 + 2 * k:6 + 2 * k], s1=-3.0e38,
                                  imm2=1.0,
                                  accum_out=tri_acc[:, 2 * k + 1:2 * k + 2])

        tri_mms(0)
        tri_mms(1)
        wsum = sing.tile([128, 1], f32)
        wsum16 = sing.tile([128, 1], bf16)

        # ---------------- main exp loop
        acta = sing.tile([128, NBT, len(ACT_GROUPS)], f32)
        accd = sing.tile([128, NBT], f32)
        dve_w = (13 - NA) * 512

        tri_work = {5: lambda: tri_hp(0), 7: lambda: tri_hn(0),
                    9: lambda: tri_hp(1), 11: lambda: tri_hn(1)}

        STAG = 3
        for b in range(NBT + STAG):
            if b <= NBT - 1:
                bt = b
                lhs = embTs[:, 128 * bt:128 * bt + 128]
                m = 0
                for gi, gsz in enumerate(ACT_GROUPS):
                    pa = psA.tile([128, 1536], f32, tag="pa")
                    for j in range(gsz):
                        nc.tensor.matmul(pa[:, 512 * j:512 * (j + 1)], lhs,
                                         wTn[:, 512 * (m + j):512 * (m + j + 1)],
                                         start=True, stop=True)
                    aj = tmp.tile([128, 1536], bf16, tag="aj")
                    nc.scalar.activation(out=aj[:, :512 * gsz],
                                         in_=pa[:, :512 * gsz],
                                         func=AF.Exp, scale=float(1.0 / A_H),
                                         accum_out=acta[:, bt, gi:gi + 1])
                    m += gsz
            if b == 0:
                wprep_group(*WG[2])
            if b == 1:
                wprep_group(*WG[3])
            if b == 2:
                wj = tmp.tile([128, CPAD], bf16, tag="wj", bufs=1)
                nc.vector.scalar_tensor_tensor(out=wj, in0=wTn, scalar=1.0,
                                               op0=A.mult, in1=wTn, op1=A.max,
                                               accum_out=wsum[:, 0:1])
                nc.vector.tensor_copy(out=wsum16, in_=wsum)
            if STAG <= b <= NBT - 1 + STAG:
                bt = b - STAG
                lhs = embTs[:, 128 * bt:128 * bt + 128]
                m = NA
                n16 = n16p.tile([128, dve_w], i16, tag="n16")
                off = 0
                for gsz in DVE_GROUPS:
                    pd = psD.tile([128, 1024], f32, tag="pd")
                    for j in range(gsz):
                        nc.tensor.matmul(pd[:, 512 * j:512 * (j + 1)], lhs,
                                         wTn[:, 512 * (m + j):512 * (m + j + 1)],
                                         start=True, stop=True)
                    nc.vector.tensor_scalar(out=n16[:, off:off + 512 * gsz],
                                            in0=pd[:, :512 * gsz],
                                            scalar1=float(S1),
                                            scalar2=0.0, op0=A.add, op1=A.max)
                    m += gsz
                    off += 512 * gsz
                junk = tmp.tile([128, dve_w], fp16, tag="junk")
                nv = n16.bitcast(fp16)
                nc.vector.scalar_tensor_tensor(out=junk, in0=nv, scalar=65504.0,
                                               op0=A.min, in1=nv, op1=A.min,
                                               accum_out=accd[:, bt:bt + 1])
            w = tri_work.get(b)
            if w is not None:
                w()

        # ---------------- tail: sumcos + se combine + outputs
        psc = psD.tile([128, 1024], f32, tag="pd")
        for bt in range(NBT):
            nc.tensor.matmul(psc[:, bt:bt + 1], embT[:, 128 * bt:128 * bt + 128],
                             wsum16, start=True, stop=True)
        sc = sing.tile([128, NBT], f32)
        nc.vector.tensor_tensor(out=sc, in0=psc[:, :NBT], in1=rinv16, op=A.mult)
        nc.sync.dma_start(out=o_sc, in_=sc)

        seA = sing.tile([128, NBT], f32)
        nc.vector.tensor_reduce(out=seA, in_=acta, axis=X, op=A.add)
        sed = sing.tile([128, NBT], f32)
        nc.vector.tensor_scalar(out=sed, in0=accd, scalar1=float(EDELTA),
                                scalar2=None, op0=A.mult)
        nc.vector.tensor_tensor(out=seA, in0=seA, in1=sed, op=A.add)
        nc.sync.dma_start(out=o_se, in_=seA)
        nc.sync.dma_start(out=o_tri, in_=tri_acc)

    nc.compile()
    return nc


def _get_nc():
    if "nc" not in _CACHE:
        _CACHE["nc"] = _build_nc()
    return _CACHE["nc"]


def _prep(embeddings, arcface_weight_mat, labels):
    bf = ml_dtypes.bfloat16
    emb = np.ascontiguousarray(embeddings, dtype=np.float32)
    W = np.ascontiguousarray(arcface_weight_mat, dtype=np.float32)
    lab = np.asarray(labels).astype(np.int64)

    order = np.argsort(lab, kind="stable")
    emb_s = emb[order]
    lab_s = lab[order]
    starts = np.searchsorted(lab_s, lab_s, side="left").astype(np.int64)
    ends = np.searchsorted(lab_s, lab_s, side="right").astype(np.int64)
    counts = ends - starts

    ss = np.einsum("bd,bd->b", emb_s, emb_s, dtype=np.float64)
    rinv = (1.0 / np.sqrt(ss)).astype(np.float32)
    embTs = np.ascontiguousarray(
        (emb_s * (A_H * ARC_SCALE * rinv)[:, None]).T).astype(bf)
    embT = np.ascontiguousarray(emb_s.T).astype(bf)
    rinv16 = np.ascontiguousarray(rinv.reshape(NBT, 128).T)

    wlab = W[lab_s].astype(np.float64)
    dots = np.einsum("bd,bd->b", emb_s.astype(np.float64), wlab)
    cl = dots * rinv.astype(np.float64) / np.linalg.norm(wlab, axis=1)
    sine = np.sqrt(np.clip(1.0 - cl * cl, 0.0, 1.0))
    phi = cl * COS_M - sine * SIN_M
    phi = np.where(cl > TH, phi, cl - MMc)

    ss32 = ss.astype(np.float32)
    in_maps = []
    for c in range(NCORES):
        wshard = np.zeros((CPAD, D), np.float32)
        wshard[:CSH] = W[c * CSH:(c + 1) * CSH]
        rows = slice(c * RB, (c + 1) * RB)
        embBT2 = np.ascontiguousarray((-2.0 * emb_s[rows]).T).astype(bf)
        ssB = np.ascontiguousarray(ss32[rows].reshape(2, 128).T)
        # per-core column permutation: own neighborhood (wrapped) first
        W0 = c * RB - 64
        head = (W0 + np.arange(384)) % B
        inhead = np.zeros(B, bool)
        inhead[head] = True
        perm = np.concatenate([head, np.nonzero(~inhead)[0]])
        embTt = np.ascontiguousarray(emb_s[perm].T).astype(bf)
        ssbt = ss32[perm].astype(bf)
        st_p = starts[rows] - W0          # positions in perm space (band in head)
        en_p = ends[rows] - W0
        assert st_p.min() >= 0 and en_p.max() <= 384, "label band escaped head"
        st2 = st_p.reshape(2, 128)
        en2 = en_p.reshape(2, 128)
        # hp windows: chunk k window = perm cols [128k, 128k+256)
        cols = [st2[0] - 0, en2[0] - 0, st2[1] - 128, en2[1] - 128,
                st2[0], en2[0], st2[1], en2[1]]
        assert cols[0].min() >= 0 and cols[1].max() <= 256
        assert cols[2].min() >= 0 and cols[3].max() <= 256
        stend = np.ascontiguousarray(
            np.stack(cols, axis=1).astype(np.float32))
        in_maps.append({
            "wsh": wshard.astype(bf),
            "embTs": embTs, "embT": embT, "embTt": embTt, "embBT2": embBT2,
            "ssbt": ssbt, "ssB": ssB, "stend": stend, "rinv16": rinv16,
        })
    host = {"ss": ss, "cl": cl, "phi": phi, "counts": counts}
    return in_maps, host


def _combine(results, host):
    s = ARC_SCALE
    S = np.zeros(B, np.float64)
    Csum = np.zeros(B, np.float64)
    for r in results:
        S += r["sumexp"].T.reshape(-1).astype(np.float64)
        Csum += r["sumcos"].T.reshape(-1).astype(np.float64)
    cl, phi = host["cl"], host["phi"]
    S += np.exp(s * phi) - np.exp(s * cl)
    Csum += phi - cl
    lse = np.log(S)
    nll = lse - s * phi
    smooth = lse - s * Csum / C
    arc = np.mean((1.0 - LABEL_SMOOTH) * nll + LABEL_SMOOTH * smooth)

    hp2 = np.concatenate([np.stack([r["tri"][:, 0], r["tri"][:, 2]], 1).T.reshape(-1)
                          for r in results])
    hn2raw = np.concatenate([np.stack([r["tri"][:, 1], r["tri"][:, 3]], 1).T.reshape(-1)
                             for r in results])
    ssv = host["ss"]
    hp = np.sqrt(np.clip(hp2 + ssv, 0, None) + 1e-16)
    hn = np.sqrt(np.clip(ssv - hn2raw, 0, None) + 1e-16)
    lossv = np.maximum(hp - hn + TRIPLET_MARGIN, 0.0)
    valid = (host["counts"] > 1).astype(np.float64)
    nv = valid.sum()
    tri = float((lossv * valid).sum() / max(nv, 1.0)) if nv > 0 else 0.0
    return np.array(W_ARC * arc + W_TRI * tri, dtype=np.float32)


def run_kernel(embeddings, arcface_weight_mat, labels, trace=False):
    from concourse.bass_utils import run_bass_kernel_spmd

    nc = _get_nc()
    in_maps, host = _prep(embeddings, arcface_weight_mat, labels)
    res = run_bass_kernel_spmd(nc, in_maps, list(range(NCORES)), trace=trace)
    return _combine(res.results, host), res


def kernel(embeddings, arcface_weight_mat, labels):
    out, _ = run_kernel(embeddings, arcface_weight_mat, labels)
    return out


# revision 14
# speedup vs baseline: 1.3498x; 1.3498x over previous
"""Fused ArcFace + batch-hard-triplet loss on 8 TRN2 NeuronCores — v10.

Per core (class-shard 6656-padded, batch sorted by label on host):
  - exp drain: ScalarE native exp (accum_out) for NA=9 tiles/bt; DVE
    fp16-Schraudolph for 4 tiles/bt (TS add+max -> int16, then STT
    min/min accum over the fp16-bitcast view at 2x).
  - W normalized on device (52 STT norms + Newton rsqrt + batched TT
    scale) then ONE batched SBUF->SBUF DMA block-transpose.
  - triplet: labels sorted + per-core column permutation (own-neighborhood
    first, wrapped) makes same-label bands static windows ->
    TENSOR_MASK_REDUCE windows for hardest-pos, full-width inverted for
    hardest-neg.
"""
import math
import os
import sys
from contextlib import ExitStack

import numpy as np

for _p in ("/opt/trn_rl_repo", os.path.expanduser("~/.axon_site/_ro/trn_rl_repo")):
    if _p not in sys.path and os.path.isdir(_p):
        sys.path.insert(0, _p)

import ml_dtypes

B, D, C = 2048, 128, 50000
NCORES = 8
CSH = C // NCORES            # 6250
CPAD = 6656                  # 52 tiles of 128 = 13 x 512
NWT = CPAD // 128            # 52
NBT = B // 128               # 16
RB = B // NCORES             # 256
ARC_SCALE = 64.0
ARC_MARGIN = 0.5
COS_M, SIN_M = math.cos(ARC_MARGIN), math.sin(ARC_MARGIN)
TH = math.cos(math.pi - ARC_MARGIN)
MMc = math.sin(math.pi - ARC_MARGIN) * ARC_MARGIN
LABEL_SMOOTH = 0.1
TRIPLET_MARGIN = 0.3
W_ARC, W_TRI = 1.0, 0.5

A_H = 2.0 ** 10 / math.log(2.0)
DELTA = 18.0
C_H = 60.0
S1 = 15.0 * 1024 - C_H - A_H * DELTA
EDELTA = math.exp(DELTA)

NA = int(os.environ.get("KERNEL_NA", "10"))          # ACT tiles per bt (of 13)
ACT_GROUPS = {7: [3, 2, 2], 8: [3, 3, 2], 9: [3, 3, 3], 10: [3, 3, 2, 2]}[NA]
DVE_GROUPS = {3: [2, 1], 4: [2, 2], 5: [2, 2, 1], 6: [2, 2, 2]}[13 - NA]

_CACHE = {}


def _build_nc():
    import concourse.bass as bass
    from concourse import bacc, mybir, tile
    from concourse.dve_ops import TENSOR_MASK_REDUCE

    f32 = mybir.dt.float32
    bf16 = mybir.dt.bfloat16
    fp16 = mybir.dt.float16
    i16 = mybir.dt.int16
    A = mybir.AluOpType
    AF = mybir.ActivationFunctionType
    X = mybir.AxisListType.X

    nc = bacc.Bacc("TRN2", target_bir_lowering=False, debug=False,
                   num_devices=NCORES)

    wsh = nc.dram_tensor("wsh", [CPAD, D], bf16, kind="ExternalInput").ap()
    embTs_d = nc.dram_tensor("embTs", [128, B], bf16, kind="ExternalInput").ap()
    embT_d = nc.dram_tensor("embT", [128, B], bf16, kind="ExternalInput").ap()
    embTt_d = nc.dram_tensor("embTt", [128, B], bf16, kind="ExternalInput").ap()
    embBT2_d = nc.dram_tensor("embBT2", [128, RB], bf16, kind="ExternalInput").ap()
    ssbt_d = nc.dram_tensor("ssbt", [B], bf16, kind="ExternalInput").ap()
    ssB_d = nc.dram_tensor("ssB", [128, 2], f32, kind="ExternalInput").ap()
    stend_d = nc.dram_tensor("stend", [128, 8], f32, kind="ExternalInput").ap()
    rinv_d = nc.dram_tensor("rinv16", [128, NBT], f32, kind="ExternalInput").ap()

    o_se = nc.dram_tensor("sumexp", [128, NBT], f32, kind="ExternalOutput").ap()
    o_sc = nc.dram_tensor("sumcos", [128, NBT], f32, kind="ExternalOutput").ap()
    o_tri = nc.dram_tensor("tri", [128, 4], f32, kind="ExternalOutput").ap()

    with tile.TileContext(nc) as tc, ExitStack() as ctx:
        sing = ctx.enter_context(tc.tile_pool(name="sing", bufs=1))
        tmp = ctx.enter_context(tc.tile_pool(name="tmp", bufs=2))
        n16p = ctx.enter_context(tc.tile_pool(name="n16p", bufs=2))
        psA = ctx.enter_context(tc.tile_pool(name="psA", bufs=2, space="PSUM"))
        psD = ctx.enter_context(tc.tile_pool(name="psD", bufs=1, space="PSUM"))

        # ---------------- input DMAs
        embTs = sing.tile([128, B], bf16)
        nc.sync.dma_start(out=embTs, in_=embTs_d)
        embT = sing.tile([128, B], bf16)
        nc.sync.dma_start(out=embT, in_=embT_d)
        embTt = sing.tile([128, B], bf16)
        nc.sync.dma_start(out=embTt, in_=embTt_d)
        embBT2 = sing.tile([128, RB], bf16)
        nc.sync.dma_start(out=embBT2, in_=embBT2_d)
        SQB = sing.tile([128, B], bf16)
        nc.sync.dma_start(out=SQB, in_=ssbt_d.partition_broadcast(128))
        ssB = sing.tile([128, 2], f32)
        nc.sync.dma_start(out=ssB, in_=ssB_d)
        stend = sing.tile([128, 8], f32)
        nc.sync.dma_start(out=stend, in_=stend_d)
        rinv16 = sing.tile([128, NBT], f32)
        nc.sync.dma_start(out=rinv16, in_=rinv_d)

        # ---------------- W prep, pipelined in 4 tile-groups
        WG = [(0, 18), (18, 18), (36, 8), (44, 8)]
        wnat = sing.tile([128, NWT, 128], bf16)
        wsrc = wsh.rearrange("(t p) d -> p t d", p=128)
        wn = sing.tile([128, NWT, 128], bf16)
        wTn = sing.tile([128, CPAD], bf16)
        ssw = sing.tile([128, NWT], f32)
        rw = sing.tile([128, NWT], f32)

        def wprep_group(t0, gn):
            sl = slice(t0, t0 + gn)
            nc.sync.dma_start(out=wnat[:, sl, :], in_=wsrc[:, sl, :])
            sq = tmp.tile([128, 18, 128], bf16, tag="sq")
            nc.vector.tensor_tensor(out=sq[:, :gn, :], in0=wnat[:, sl, :],
                                    in1=wnat[:, sl, :], op=A.mult)
            nc.vector.tensor_reduce(out=ssw[:, sl], in_=sq[:, :gn, :], axis=X,
                                    op=A.add)
            nc.vector.memset(rw[:, sl], 14.0)
            for it in range(3):
                t1 = tmp.tile([128, 18], f32, tag="nt1")
                nc.vector.tensor_tensor(out=t1[:, :gn], in0=rw[:, sl],
                                        in1=rw[:, sl], op=A.mult)
                nc.vector.tensor_tensor(out=t1[:, :gn], in0=t1[:, :gn],
                                        in1=ssw[:, sl], op=A.mult)
                nc.vector.tensor_scalar(out=t1[:, :gn], in0=t1[:, :gn],
                                        scalar1=-0.5, scalar2=1.5,
                                        op0=A.mult, op1=A.add)
                nc.vector.tensor_tensor(out=rw[:, sl], in0=rw[:, sl],
                                        in1=t1[:, :gn], op=A.mult)
            rwb = rw[:, sl].to_broadcast((128, gn, 128))
            nc.vector.tensor_tensor(out=wn[:, sl, :], in0=wnat[:, sl, :],
                                    in1=rwb, op=A.mult)
            nc.sync.dma_start_transpose(
                out=wTn[:, 128 * t0:128 * (t0 + gn)].rearrange(
                    "a (t p) -> a t p", p=128),
                in_=wn[:, sl, :].rearrange("a t d -> a (t d)"))

        wprep_group(*WG[0])
        wprep_group(*WG[1])

        # ---------------- triplet (own-neighborhood-permuted columns)
        d2p = [sing.tile([128, B], bf16, name=f"d2p{k}") for k in range(2)]
        d2n = [sing.tile([128, B], bf16, name=f"d2n{k}") for k in range(2)]
        mrj = sing.tile([128, B], bf16)
        mrw = sing.tile([128, 256], bf16)
        tri_acc = sing.tile([128, 4], f32)

        def tri_mms(k):
            pa = psA.tile([128, 1536], f32, tag="pa")
            for j in range(3):
                nc.tensor.matmul(pa[:, 512 * j:512 * j + 512],
                                 embBT2[:, 128 * k:128 * k + 128],
                                 embTt[:, 512 * j:512 * j + 512],
                                 start=True, stop=True)
            pd = psD.tile([128, 1024], f32, tag="pd")
            nc.tensor.matmul(pd[:, 0:512], embBT2[:, 128 * k:128 * k + 128],
                             embTt[:, 1536:2048], start=True, stop=True)
            nc.vector.scalar_tensor_tensor(out=d2p[k][:, :1536], in0=pa,
                                           scalar=1.0, in1=SQB[:, :1536],
                                           op0=A.mult, op1=A.add)
            nc.vector.scalar_tensor_tensor(out=d2p[k][:, 1536:], in0=pd[:, 0:512],
                                           scalar=1.0, in1=SQB[:, 1536:],
                                           op0=A.mult, op1=A.add)
            nc.vector.tensor_scalar(out=d2n[k], in0=d2p[k], scalar1=-1.0,
                                    scalar2=None, op0=A.mult)

        def tri_hp(k):
            # hp^2 - ss_k = max over window [st,en) of d2pos; window = [128k, 128k+256)
            nc.vector._custom_dve(TENSOR_MASK_REDUCE, out=mrw,
                                  in0=d2p[k][:, 128 * k:128 * k + 256],
                                  in1=stend[:, 2 * k + 1:2 * k + 2],
                                  s0=stend[:, 2 * k:2 * k + 1], s1=-3.0e38,
                                  imm2=1.0, accum_out=tri_acc[:, 2 * k:2 * k + 1])

        def tri_hn(k):
            # ss_k - hn^2 = max over NOT [st,en) of -d2pos (global perm coords)
            nc.vector._custom_dve(TENSOR_MASK_REDUCE, out=mrj, in0=d2n[k],
                                  in1=stend[:, 4 + 2 * k:5 + 2 * k],
                                  s0=stend[:, 5 + 2 * k:6 + 2 * k], s1=-3.0e38,
                                  imm2=1.0,
                                  accum_out=tri_acc[:, 2 * k + 1:2 * k + 2])

        tri_mms(0)
        tri_mms(1)
        wsum = sing.tile([128, 1], f32)
        wsum16 = sing.tile([128, 1], bf16)

        # ---------------- main exp loop
        acta = sing.tile([128, NBT, len(ACT_GROUPS)], f32)
        accd = sing.tile([128, NBT], f32)
        dve_w = (13 - NA) * 512

        tri_work = {5: lambda: tri_hp(0), 7: lambda: tri_hn(0),
                    9: lambda: tri_hp(1), 11: lambda: tri_hn(1)}

        STAG = 3
        for b in range(NBT + STAG):
            if b <= NBT - 1:
                bt = b
                lhs = embTs[:, 128 * bt:128 * bt + 128]
                m = 0
                for gi, gsz in enumerate(ACT_GROUPS):
                    pa = psA.tile([128, 1536], f32, tag="pa")
                    for j in range(gsz):
                        nc.tensor.matmul(pa[:, 512 * j:512 * (j + 1)], lhs,
                                         wTn[:, 512 * (m + j):512 * (m + j + 1)],
                                         start=True, stop=True)
                    aj = tmp.tile([128, 1536], bf16, tag="aj")
                    nc.scalar.activation(out=aj[:, :512 * gsz],
                                         in_=pa[:, :512 * gsz],
                                         func=AF.Exp, scale=float(1.0 / A_H),
                                         accum_out=acta[:, bt, gi:gi + 1])
                    m += gsz
            if b == 0:
                wprep_group(*WG[2])
            if b == 1:
                wprep_group(*WG[3])
            if b == 2:
                wj = tmp.tile([128, CPAD], bf16, tag="wj", bufs=1)
                nc.vector.scalar_tensor_tensor(out=wj, in0=wTn, scalar=1.0,
                                               op0=A.mult, in1=wTn, op1=A.max,
                                               accum_out=wsum[:, 0:1])
                nc.vector.tensor_copy(out=wsum16, in_=wsum)
            if STAG <= b <= NBT - 1 + STAG:
                bt = b - STAG
                lhs = embTs[:, 128 * bt:128 * bt + 128]
                m = NA
                n16 = n16p.tile([128, dve_w], i16, tag="n16")
                off = 0
                for gsz in DVE_GROUPS:
                    pd = psD.tile([128, 1024], f32, tag="pd")
                    for j in range(gsz):
                        nc.tensor.matmul(pd[:, 512 * j:512 * (j + 1)], lhs,
                                         wTn[:, 512 * (m + j):512 * (m + j + 1)],
                                         start=True, stop=True)
                    nc.vector.tensor_scalar(out=n16[:, off:off + 512 * gsz],
                                            in0=pd[:, :512 * gsz],
                                            scalar1=float(S1),
                                            scalar2=0.0, op0=A.add, op1=A.max)
                    m += gsz
                    off += 512 * gsz
                junk = tmp.tile([128, dve_w], fp16, tag="junk")
                nv = n16.bitcast(fp16)
                nc.vector.scalar_tensor_tensor(out=junk, in0=nv, scalar=65504.0,
                                               op0=A.min, in1=nv, op1=A.min,
                                               accum_out=accd[:, bt:bt + 1])
            w = tri_work.get(b)
            if w is not None:
                w()

        # ---------------- tail: sumcos + se combine + outputs
        psc = psD.tile([128, 1024], f32, tag="pd")
        for bt in range(NBT):
            nc.tensor.matmul(psc[:, bt:bt + 1], embT[:, 128 * bt:128 * bt + 128],
                             wsum16, start=True, stop=True)
        sc = sing.tile([128, NBT], f32)
        nc.vector.tensor_tensor(out=sc, in0=psc[:, :NBT], in1=rinv16, op=A.mult)
        nc.sync.dma_start(out=o_sc, in_=sc)

        seA = sing.tile([128, NBT], f32)
        nc.vector.tensor_reduce(out=seA, in_=acta, axis=X, op=A.add)
        sed = sing.tile([128, NBT], f32)
        nc.vector.tensor_scalar(out=sed, in0=accd, scalar1=float(EDELTA),
                                scalar2=None, op0=A.mult)
        nc.vector.tensor_tensor(out=seA, in0=seA, in1=sed, op=A.add)
        nc.sync.dma_start(out=o_se, in_=seA)
        nc.sync.dma_start(out=o_tri, in_=tri_acc)

    nc.compile()
    return nc


def _get_nc():
    if "nc" not in _CACHE:
        _CACHE["nc"] = _build_nc()
    return _CACHE["nc"]


def _prep(embeddings, arcface_weight_mat, labels):
    bf = ml_dtypes.bfloat16
    emb = np.ascontiguousarray(embeddings, dtype=np.float32)
    W = np.ascontiguousarray(arcface_weight_mat, dtype=np.float32)
    lab = np.asarray(labels).astype(np.int64)

    order = np.argsort(lab, kind="stable")
    emb_s = emb[order]
    lab_s = lab[order]
    starts = np.searchsorted(lab_s, lab_s, side="left").astype(np.int64)
    ends = np.searchsorted(lab_s, lab_s, side="right").astype(np.int64)
    counts = ends - starts

    ss = np.einsum("bd,bd->b", emb_s, emb_s, dtype=np.float64)
    rinv = (1.0 / np.sqrt(ss)).astype(np.float32)
    embTs = np.ascontiguousarray(
        (emb_s * (A_H * ARC_SCALE * rinv)[:, None]).T).astype(bf)
    embT = np.ascontiguousarray(emb_s.T).astype(bf)
    rinv16 = np.ascontiguousarray(rinv.reshape(NBT, 128).T)

    wlab = W[lab_s].astype(np.float64)
    dots = np.einsum("bd,bd->b", emb_s.astype(np.float64), wlab)
    cl = dots * rinv.astype(np.float64) / np.linalg.norm(wlab, axis=1)
    sine = np.sqrt(np.clip(1.0 - cl * cl, 0.0, 1.0))
    phi = cl * COS_M - sine * SIN_M
    phi = np.where(cl > TH, phi, cl - MMc)

    ss32 = ss.astype(np.float32)
    in_maps = []
    for c in range(NCORES):
        wshard = np.zeros((CPAD, D), np.float32)
        wshard[:CSH] = W[c * CSH:(c + 1) * CSH]
        rows = slice(c * RB, (c + 1) * RB)
        embBT2 = np.ascontiguousarray((-2.0 * emb_s[rows]).T).astype(bf)
        ssB = np.ascontiguousarray(ss32[rows].reshape(2, 128).T)
        # per-core column permutation: own neighborhood (wrapped) first
        W0 = c * RB - 64
        head = (W0 + np.arange(384)) % B
        inhead = np.zeros(B, bool)
        inhead[head] = True
        perm = np.concatenate([head, np.nonzero(~inhead)[0]])
        embTt = np.ascontiguousarray(emb_s[perm].T).astype(bf)
        ssbt = ss32[perm].astype(bf)
        st_p = starts[rows] - W0          # positions in perm space (band in head)
        en_p = ends[rows] - W0
        assert st_p.min() >= 0 and en_p.max() <= 384, "label band escaped head"
        st2 = st_p.reshape(2, 128)
        en2 = en_p.reshape(2, 128)
        # hp windows: chunk k window = perm cols [128k, 128k+256)
        cols = [st2[0] - 0, en2[0] - 0, st2[1] - 128, en2[1] - 128,
                st2[0], en2[0], st2[1], en2[1]]
        assert cols[0].min() >= 0 and cols[1].max() <= 256
        assert cols[2].min() >= 0 and cols[3].max() <= 256
        stend = np.ascontiguousarray(
            np.stack(cols, axis=1).astype(np.float32))
        in_maps.append({
            "wsh": wshard.astype(bf),
            "embTs": embTs, "embT": embT, "embTt": embTt, "embBT2": embBT2,
            "ssbt": ssbt, "ssB": ssB, "stend": stend, "rinv16": rinv16,
        })
    host = {"ss": ss, "cl": cl, "phi": phi, "counts": counts}
    return in_maps, host


def _combine(results, host):
    s = ARC_SCALE
    S = np.zeros(B, np.float64)
    Csum = np.zeros(B, np.float64)
    for r in results:
        S += r["sumexp"].T.reshape(-1).astype(np.float64)
        Csum += r["sumcos"].T.reshape(-1).astype(np.float64)
    cl, phi = host["cl"], host["phi"]
    S += np.exp(s * phi) - np.exp(s * cl)
    Csum += phi - cl
    lse = np.log(S)
    nll = lse - s * phi
    smooth = lse - s * Csum / C
    arc = np.mean((1.0 - LABEL_SMOOTH) * nll + LABEL_SMOOTH * smooth)

    hp2 = np.concatenate([np.stack([r["tri"][:, 0], r["tri"][:, 2]], 1).T.reshape(-1)
                          for r in results])
    hn2raw = np.concatenate([np.stack([r["tri"][:, 1], r["tri"][:, 3]], 1).T.reshape(-1)
                             for r in results])
    ssv = host["ss"]
    hp = np.sqrt(np.clip(hp2 + ssv, 0, None) + 1e-16)
    hn = np.sqrt(np.clip(ssv - hn2raw, 0, None) + 1e-16)
    lossv = np.maximum(hp - hn + TRIPLET_MARGIN, 0.0)
    valid = (host["counts"] > 1).astype(np.float64)
    nv = valid.sum()
    tri = float((lossv * valid).sum() / max(nv, 1.0)) if nv > 0 else 0.0
    return np.array(W_ARC * arc + W_TRI * tri, dtype=np.float32)


def run_kernel(embeddings, arcface_weight_mat, labels, trace=False):
    from concourse.bass_utils import run_bass_kernel_spmd

    nc = _get_nc()
    in_maps, host = _prep(embeddings, arcface_weight_mat, labels)
    res = run_bass_kernel_spmd(nc, in_maps, list(range(NCORES)), trace=trace)
    return _combine(res.results, host), res


def kernel(embeddings, arcface_weight_mat, labels):
    out, _ = run_kernel(embeddings, arcface_weight_mat, labels)
    return out
